# revision 35
# baseline (speedup 1.0000x reference)
"""Trainium2 Bass kernel for nn_CrossAttention (N=65536 gaussians, M=512 tokens, D=512).

Runs SPMD on 8 NeuronCores; N sharded across cores (n_loc=8192 rows each).

v2 design (vs v1 baseline at 679us):
  - Host precomputes all weight-derived matrices (V, aq0, agp0, GQ, GGP, c0, u0)
    and ships g/g_p pre-transposed, removing all device-side weight prep and
    all PE transposes.
  - gT is DMA'd once into a persistent SBUF stash (f32r, 128KB/partition) and
    reused by both the pooling pass and the attention pass.
  - Pooling accumulation (P = gp.T @ E, l = 1.T @ E) runs in bf16; the
    AllReduce payload [128, 2560] is bf16 (P.T tiles + l replicated).
  - LayerNorm rstd = Exp(-0.5 * Ln(var+eps)) keeps every ACT func in one
    activation table (no LoadActFuncSet thrash); final scale/shift happens on
    ACT via Identity(bias=-mu*rstd, scale=rstd).
  - Residual+scale fused into one DVE scalar_tensor_tensor writing PSUM
    in place.

Math (per core, n_loc rows):
  E = exp(g @ Ws.T - C_SHIFT)             [n, m]   (pool scores, fixed shift)
  P.T = gp.T @ E, l = 1.T @ E             -> AllReduce -> kplT = P.T / l
  aq_s = aq0 + GQ.T-contract kplT  (aq0 = SCALE*Wq.T@K0.T, K0 = Ws@Wk.T+bk+bkp)
  scoresT = aq_s.T-contract gT + agp_s.T-contract gpT   [m, n]
  ep = exp(scoresT + c),   c = c0 + u0-contract kplT
  OV = ep.T-contract V ; r = col-sums ; out = LN(OV/r + g) * gamma + beta
"""
import numpy as np
import ml_dtypes

import concourse.bass as bass
import concourse.tile as tile
from concourse import bacc, mybir, bass_utils


N_CORES = 8
N_FULL = 65536
D = 512
M = 512
SCALE = (D // 8) ** -0.5  # 0.125
LN_EPS = 1e-5
C_SHIFT = 115.0
Z_CHUNKS = 5   # chunks whose scores are prefactored (scores0 + Z) to fill the
               # AllReduce window with AR-independent PE work
F32 = mybir.dt.float32
F32R = mybir.dt.float32r
BF16 = mybir.dt.bfloat16
F16 = mybir.dt.float16
EXP = mybir.ActivationFunctionType.Exp
LN_F = mybir.ActivationFunctionType.Ln
SQRT = mybir.ActivationFunctionType.Sqrt
IDENT = mybir.ActivationFunctionType.Identity
MULT = mybir.AluOpType.mult
ADD = mybir.AluOpType.add


def _bcast(ap, parts):
    """Partition-broadcast a [F]-shaped DRAM AP to [parts, F] for DMA."""
    return bass.AP(tensor=ap.tensor, offset=ap.offset, ap=[[0, parts], *ap.ap])


def build(n_loc=N_FULL // N_CORES, n_cores=N_CORES, use_gb=False, debug=False):
    nch = n_loc // 512     # chunks of 512 rows
    assert n_loc % 512 == 0

    nc = bacc.Bacc("TRN2", target_bir_lowering=False, debug=False,
                   num_devices=n_cores)
    # per-core sharded inputs
    gt_d = nc.dram_tensor("gt", [D, n_loc], F16, kind="ExternalInput").ap()
    gpt_d = nc.dram_tensor("gpt", [D, n_loc], F16, kind="ExternalInput").ap()
    gpn_d = nc.dram_tensor("gpn", [n_loc, D], F16, kind="ExternalInput").ap()
    gn_d = nc.dram_tensor("gn", [n_loc, D], F16, kind="ExternalInput").ap()
    # replicated (host-precomputed) weights
    wst_d = nc.dram_tensor("wst", [D, M], F32R, kind="ExternalInput").ap()
    v_d = nc.dram_tensor("v", [M, D], BF16, kind="ExternalInput").ap()
    aq0_d = nc.dram_tensor("aq0", [D, M], F16, kind="ExternalInput").ap()
    agp0_d = nc.dram_tensor("agp0", [D, M], F16, kind="ExternalInput").ap()
    gq_d = nc.dram_tensor("gq", [D, D], BF16, kind="ExternalInput").ap()
    ggp_d = nc.dram_tensor("ggp", [D, D], BF16, kind="ExternalInput").ap()
    gqt_d = nc.dram_tensor("gqt", [D, D], F16, kind="ExternalInput").ap()
    ggpt_d = nc.dram_tensor("ggpt", [D, D], F16, kind="ExternalInput").ap()
    id_d = nc.dram_tensor("id128", [128, 128], F16, kind="ExternalInput").ap()
    c0_d = nc.dram_tensor("c0", [M], F32, kind="ExternalInput").ap()
    u0_d = nc.dram_tensor("u0", [D], F32, kind="ExternalInput").ap()
    if use_gb:
        gam_d = nc.dram_tensor("gamma", [D], F32, kind="ExternalInput").ap()
        bet_d = nc.dram_tensor("beta", [D], F32, kind="ExternalInput").ap()
    out_d = nc.dram_tensor("out", [n_loc, D], F32, kind="ExternalOutput").ap()
    dbg = {}
    if debug:
        for nm, sh, dt in [("d_kplT", [128, 4, 512], F16),
                           ("d_aq", [128, 4, 512], F16),
                           ("d_agp", [128, 4, 512], F16), ("d_c", [128, 4], F32),
                           ("d_ep00", [128, 512], BF16), ("d_r0", [128, 4], F32),
                           ("d_pre00", [128, 512], F32),
                           ("d_sc00", [128, 512], F32),
                           ("d_ov00", [128, 512], F32),
                           ("d_rr0", [128, 4], F32),
                           ("d_ep01", [128, 512], BF16),
                           ("d_ep02", [128, 512], BF16),
                           ("d_ep03", [128, 512], BF16)]:
            dbg[nm] = nc.dram_tensor(nm, sh, dt, kind="ExternalOutput").ap()

    with tile.TileContext(nc) as tc:
        with (
            tc.tile_pool(name="wts", bufs=1) as wts,
            tc.tile_pool(name="ps", bufs=2, space="PSUM") as pps,
            tc.tile_pool(name="dram", bufs=1, space="DRAM") as dpool,
        ):
            # ---------- persistent tiles ----------
            K = min(Z_CHUNKS, nch)
            gstash = wts.tile([128, 4, n_loc], F16)    # g.T stash [d%128, d//128, n]
            v_sb = wts.tile([128, 4, D], BF16)         # V [m-part, mt, d]
            aq_s = wts.tile([128, 4, D], F16)          # SCALE*Wq.T@K.T [d, dt, m]
            agp_s = wts.tile([128, 4, D], F16)
            kplT = wts.tile([128, 4, 512], F16)        # pooled P.T/l [d'%128, dt, m]
            # spill space for the AR-window prework (scores0 / Z per Z-chunk)
            s0_sb = [wts.tile([128, 4, 512], F16, name=f"s0_{c}") for c in range(K)]
            z_sb = [wts.tile([128, 4, 512], F16, name=f"z_{c}") for c in range(K)]
            c0_sb = wts.tile([128, 4], F32)            # c0[m] as [m%128, mt]
            u0_sb = wts.tile([128, 4], F32)            # u0[d'] as [d'%128, dt]
            c_sb = wts.tile([128, 4], F32)
            ones_bf = wts.tile([128, 128], BF16)
            nc.vector.memset(ones_bf, 1.0)
            id_sb = wts.tile([128, 128], F16)   # identity: lets PE add a
            # [m, n]-layout tile into an open PSUM accumulation group
            eps_sb = wts.tile([128, 1], F32)
            nc.vector.memset(eps_sb, LN_EPS)
            negc_sb = wts.tile([128, 1], F32)
            nc.vector.memset(negc_sb, -C_SHIFT)
            if use_gb:
                gam_bc = wts.tile([128, D], F32)
                bet_bc = wts.tile([128, D], F32)
                nc.scalar.dma_start(out=gam_bc, in_=_bcast(gam_d, 128))
                nc.scalar.dma_start(out=bet_bc, in_=_bcast(bet_d, 128))

            # ---------- phase A: pooling partials ----------
            ctxA = tc.tile_pool(name="pA", bufs=1, space="PSUM")
            pA = ctxA.__enter__()
            ps_p = [pA.tile([128, 512], F32, tag=f"psp{i}", bufs=1,
                            name=f"ps_p{i}") for i in range(4)]
            ps_l = pA.tile([128, 512], F32, tag="psl", bufs=1, name="ps_l")
            with tc.tile_pool(name="sAw", bufs=1) as sAw:
                wst_sb = sAw.tile([128, 4, M], F32R)
                # halve the first loads so the first E matmul (dk=0) starts
                # early; finer splits lose to the ~0.65us per-DMA issue cost
                for t2 in range(2):
                    nc.sync.dma_start(
                        out=wst_sb[:, 2 * t2:2 * t2 + 2, :],
                        in_=wst_d.rearrange("(t p) m -> p t m",
                                            p=128)[:, 2 * t2:2 * t2 + 2, :])
                with (tc.tile_pool(name="sA", bufs=2) as sA,
                      tc.tile_pool(name="etp", bufs=1) as etpool):
                    for c in range(nch):
                        nsl = slice(c * 512, (c + 1) * 512)
                        if c == 0:
                            # halve chunk 0's load so the first E matmul
                            # (dk=0) starts after half the transfer
                            for t2 in range(2):
                                nc.sync.dma_start(
                                    out=gstash[:, 2 * t2:2 * t2 + 2, nsl],
                                    in_=gt_d.rearrange(
                                        "(t p) n -> p t n",
                                        p=128)[:, 2 * t2:2 * t2 + 2, nsl])
                        else:
                            nc.sync.dma_start(
                                out=gstash[:, :, nsl],
                                in_=gt_d.rearrange("(t p) n -> p t n", p=128)[:, :, nsl])
                        gpc = sA.tile([128, 4, D], F16, tag="gpc", name=f"gpc{c}")
                        nc.sync.dma_start(
                            out=gpc,
                            in_=gpn_d[nsl, :].rearrange("(j p) d -> p j d", p=128))
                        for j in range(4):
                            p_s = pps.tile([128, 512], F32, tag="s",
                                           name=f"psA{c}_{j}")
                            for dk in range(4):
                                nc.tensor.matmul(
                                    p_s[:],
                                    gstash[:, dk,
                                           c * 512 + j * 128:c * 512 + (j + 1) * 128],
                                    wst_sb[:, dk, :], start=(dk == 0), stop=(dk == 3))
                            et = etpool.tile([128, 512], F16, tag=f"et{j}",
                                         name=f"et{c}_{j}")
                            nc.scalar.activation(out=et, in_=p_s[:], func=EXP,
                                                 bias=negc_sb, scale=1.0)
                            first = (c == 0 and j == 0)
                            last = (c == nch - 1 and j == 3)
                            for d2t in range(4):
                                nc.tensor.matmul(
                                    ps_p[d2t][:],
                                    gpc[:, j, d2t * 128:(d2t + 1) * 128], et[:],
                                    start=first, stop=last)
                            nc.tensor.matmul(ps_l[:], ones_bf[:], et[:],
                                             start=first, stop=last)
                        if c == 0:
                            # weight loads deferred past the startup-critical
                            # wst + chunk-0 DMAs; needed only in phase B
                            nc.scalar.dma_start(
                                out=v_sb, in_=v_d.rearrange("(t p) d -> p t d", p=128))
                            nc.scalar.dma_start(
                                out=aq_s, in_=aq0_d.rearrange("(t p) m -> p t m", p=128))
                            nc.scalar.dma_start(
                                out=agp_s,
                                in_=agp0_d.rearrange("(t p) m -> p t m", p=128))
                            nc.scalar.dma_start(
                                out=c0_sb, in_=c0_d.rearrange("(t p) -> p t", p=128))
                            nc.scalar.dma_start(
                                out=u0_sb, in_=u0_d.rearrange("(t p) -> p t", p=128))

            # ---------- AllReduce of (P.T || l) in bf16 ----------
            # payload = P.T [128, 2048] + l [512] once (not 128x replicated):
            # 525,312 B -> (15000 + bytes/40) * 1.875 = 52.7us in the cost model
            NPL = 128 * 2048 + 512
            with tc.tile_pool(name="arp", bufs=1) as arp:
                pl_sb = arp.tile([128, 4 * 512], BF16)
                for d2t in range(4):
                    nc.scalar.copy(out=pl_sb[:, d2t * 512:(d2t + 1) * 512],
                                   in_=ps_p[d2t][:])
                l_row = arp.tile([1, 512], BF16)
                nc.scalar.copy(out=l_row, in_=ps_l[0:1, :])
                ctxA.__exit__(None, None, None)
                ar_in = dpool.tile([NPL], BF16)
                ar_out = dpool.tile([NPL], BF16, addr_space="Shared")
                nc.gpsimd.dma_start(
                    out=ar_in[0:128 * 2048].rearrange("(p f) -> p f", p=128),
                    in_=pl_sb[:])
                nc.gpsimd.dma_start(
                    out=ar_in[128 * 2048:NPL].rearrange("(p f) -> p f", p=1),
                    in_=l_row[:])
                nc.gpsimd.collective_compute(
                    "AllReduce", mybir.AluOpType.add,
                    replica_groups=[list(range(n_cores))],
                    ins=[ar_in.opt()], outs=[ar_out.opt()])

                # ---------- AR-window prework (AR-independent) ----------
                # For the first K chunks compute scores0 = aq0.T@gT + agp0.T@gpT
                # and Z = GQ@gT + GGP@gpT while the collective runs.  Post-AR
                # those chunks only need Delta = kplT.T@Z (16 mm instead of 32).
                gqt_sb = arp.tile([128, 4, D], F16)
                ggpt_sb = arp.tile([128, 4, D], F16)
                nc.sync.dma_start(out=gqt_sb,
                                  in_=gqt_d.rearrange("(t p) e -> p t e", p=128))
                nc.sync.dma_start(out=ggpt_sb,
                                  in_=ggpt_d.rearrange("(t p) e -> p t e", p=128))
                nc.sync.dma_start(out=id_sb, in_=id_d)
                with tc.tile_pool(name="zin", bufs=2) as zin:
                    for c in range(K):
                        nsl = slice(c * 512, (c + 1) * 512)
                        gptz = zin.tile([128, 4, 512], F16, tag="gptz",
                                        name=f"gptz{c}")
                        nc.sync.dma_start(
                            out=gptz,
                            in_=gpt_d.rearrange("(t p) n -> p t n", p=128)[:, :, nsl])
                        for mt in range(4):
                            p_s0 = pps.tile([128, 512], F32, tag="s",
                                            name=f"zs{c}_{mt}")
                            for dk in range(4):
                                nc.tensor.matmul(
                                    p_s0[:], aq_s[:, dk, mt * 128:(mt + 1) * 128],
                                    gstash[:, dk, nsl], start=(dk == 0), stop=False)
                            for dk in range(4):
                                nc.tensor.matmul(
                                    p_s0[:], agp_s[:, dk, mt * 128:(mt + 1) * 128],
                                    gptz[:, dk, :], start=False, stop=(dk == 3))
                            nc.scalar.copy(out=s0_sb[c][:, mt, :], in_=p_s0[:])
                        for dt in range(4):
                            p_z = pps.tile([128, 512], F32, tag="s",
                                           name=f"zz{c}_{dt}")
                            for dk in range(4):
                                nc.tensor.matmul(
                                    p_z[:], gqt_sb[:, dk, dt * 128:(dt + 1) * 128],
                                    gstash[:, dk, nsl], start=(dk == 0), stop=False)
                            for dk in range(4):
                                nc.tensor.matmul(
                                    p_z[:], ggpt_sb[:, dk, dt * 128:(dt + 1) * 128],
                                    gptz[:, dk, :], start=False, stop=(dk == 3))
                            nc.scalar.copy(out=z_sb[c][:, dt, :], in_=p_z[:])

                plr_sb = arp.tile([128, 4 * 512], BF16)
                nc.sync.dma_start(
                    out=plr_sb,
                    in_=ar_out[0:128 * 2048].rearrange("(p f) -> p f", p=128))
                lrep = arp.tile([128, 512], BF16)
                nc.sync.dma_start(out=lrep, in_=_bcast(ar_out[128 * 2048:NPL], 128))

                # ---------- post-pool fixups ----------
                lrec = arp.tile([128, 512], F32)
                nc.vector.reciprocal(out=lrec, in_=lrep)
                for dt in range(4):
                    nc.vector.tensor_mul(out=kplT[:, dt, :],
                                         in0=plr_sb[:, dt * 512:(dt + 1) * 512],
                                         in1=lrec)
                # c correction first: every ep exp needs the c bias
                u0b = arp.tile([128, 4], F16)
                nc.vector.tensor_copy(out=u0b, in_=u0_sb)
                with tc.tile_pool(name="pc", bufs=1, space="PSUM") as pcp:
                    p_c = pcp.tile([128, 4], F32, tag="pc", bufs=1, name="p_c")
                    for mt in range(4):
                        for di in range(4):
                            nc.tensor.matmul(
                                p_c[:, mt:mt + 1],
                                kplT[:, di, mt * 128:(mt + 1) * 128],
                                u0b[:, di:di + 1], start=(di == 0), stop=(di == 3))
                    nc.vector.tensor_add(out=c_sb, in0=c0_sb, in1=p_c[:])
                gq_sb = arp.tile([128, 4, D], BF16)
                ggp_sb = arp.tile([128, 4, D], BF16)
                nc.sync.dma_start(out=gq_sb,
                                  in_=gq_d.rearrange("(t p) d -> p t d", p=128))
                nc.sync.dma_start(out=ggp_sb,
                                  in_=ggp_d.rearrange("(t p) d -> p t d", p=128))
                for gmat, dst in ((gq_sb, aq_s), (ggp_sb, agp_s)):
                    for dt in range(4):
                        p_corr = pps.tile([128, 512], F32, tag="s",
                                          name=f"pc_{dst.tensor.name}_{dt}")
                        for di in range(4):
                            nc.tensor.matmul(
                                p_corr[:], gmat[:, di, dt * 128:(dt + 1) * 128],
                                kplT[:, di, :], start=(di == 0), stop=(di == 3))
                        nc.vector.tensor_add(out=dst[:, dt, :], in0=dst[:, dt, :],
                                             in1=p_corr[:])
                if debug:
                    nc.sync.dma_start(out=dbg["d_kplT"], in_=kplT)
                    nc.sync.dma_start(out=dbg["d_aq"], in_=aq_s)
                    nc.sync.dma_start(out=dbg["d_agp"], in_=agp_s)
                    nc.sync.dma_start(out=dbg["d_c"], in_=c_sb)

            # ---------- phase B: attention ----------
            with (tc.tile_pool(name="sB", bufs=3) as sB,
                  tc.tile_pool(name="epp", bufs=1) as epool,
                  tc.tile_pool(name="eB", bufs=2) as eB,
                  tc.tile_pool(name="oB", bufs=1) as oB,
                  tc.tile_pool(name="pB", bufs=1, space="PSUM") as pB):
                for c in range(nch):
                    nsl = slice(c * 512, (c + 1) * 512)
                    if c >= K:
                        gptc = sB.tile([128, 4, 512], F16, tag="gptc",
                                       name=f"gptc{c}")
                        nc.sync.dma_start(
                            out=gptc,
                            in_=gpt_d.rearrange("(t p) n -> p t n", p=128)[:, :, nsl])
                    gnc = sB.tile([128, 4, 512], F16, tag="gnc", name=f"gnc{c}")
                    nc.sync.dma_start(
                        out=gnc, in_=gn_d[nsl, :].rearrange("(j p) d -> p j d", p=128))
                    ps_ov = [pB.tile([128, 512], F32, tag=f"ov{k}", bufs=1,
                                     name=f"ov{c}_{k}") for k in range(4)]
                    ps_r = pB.tile([128, 4], F32, tag="r", bufs=1, name=f"r{c}")
                    eps = []
                    for mt in range(4):
                        p_sc = pps.tile([128, 512], F32, tag="s", name=f"sc{c}_{mt}")
                        if c < K:
                            # Delta = kplT.T @ Z, + spilled scores0 via an
                            # identity matmul (keeps the whole sum on PE)
                            for dt in range(4):
                                nc.tensor.matmul(
                                    p_sc[:], kplT[:, dt, mt * 128:(mt + 1) * 128],
                                    z_sb[c][:, dt, :], start=(dt == 0), stop=False)
                            nc.tensor.matmul(p_sc[:], id_sb[:],
                                             s0_sb[c][:, mt, :],
                                             start=False, stop=True)
                        else:
                            for dk in range(4):
                                nc.tensor.matmul(
                                    p_sc[:], aq_s[:, dk, mt * 128:(mt + 1) * 128],
                                    gstash[:, dk, nsl], start=(dk == 0), stop=False)
                            for dk in range(4):
                                nc.tensor.matmul(
                                    p_sc[:], agp_s[:, dk, mt * 128:(mt + 1) * 128],
                                    gptc[:, dk, :], start=False, stop=(dk == 3))
                        ep = epool.tile([128, 512], BF16, tag=f"ep{mt}",
                                     name=f"ep{c}_{mt}")
                        if debug and c == 0 and mt == 0:
                            dsc = eB.tile([128, 512], F32, name="dsc")
                            nc.vector.tensor_copy(out=dsc, in_=p_sc[:])
                            nc.sync.dma_start(out=dbg["d_sc00"], in_=dsc)
                        nc.scalar.activation(out=ep, in_=p_sc[:], func=EXP,
                                             bias=c_sb[:, mt:mt + 1], scale=1.0)
                        if debug and c == 0:
                            nc.sync.dma_start(out=dbg[f"d_ep0{mt}"], in_=ep)
                        eps.append(ep)
                        for nk in range(4):
                            nc.tensor.matmul(
                                ps_ov[nk][:], ep[:, nk * 128:(nk + 1) * 128],
                                v_sb[:, mt, :], start=(mt == 0), stop=(mt == 3))
                    # r groups must be column-sequential: start=True resets
                    # has_written for the whole bank, so interleaving the four
                    # column-groups of one tile drops earlier columns' first
                    # epoch.
                    for nk in range(4):
                        for mt in range(4):
                            nc.tensor.matmul(
                                ps_r[:, nk:nk + 1],
                                eps[mt][:, nk * 128:(nk + 1) * 128],
                                ones_bf[:, 0:1], start=(mt == 0), stop=(mt == 3))
                    rr = eB.tile([128, 4], F32, tag="rr", name=f"rr{c}")
                    if debug and c == 0:
                        dov = eB.tile([128, 512], F32, name="dov")
                        nc.vector.tensor_copy(out=dov, in_=ps_ov[0][:])
                        nc.sync.dma_start(out=dbg["d_ov00"], in_=dov)
                        drr = eB.tile([128, 4], F32, name="drr")
                        nc.vector.tensor_copy(out=drr, in_=ps_r[:])
                        nc.sync.dma_start(out=dbg["d_r0"], in_=drr)
                    nc.vector.reciprocal(out=rr, in_=ps_r[:])
                    if debug and c == 0:
                        nc.sync.dma_start(out=dbg["d_rr0"], in_=rr)
                    ob = oB.tile([128, 4, 512], F32, tag="ob", name=f"ob{c}")
                    od = out_d.rearrange("(c j p) d -> c p j d", p=128, j=4)[c]
                    for nk in range(4):
                        # pre = OV * (1/r) + g, in place in PSUM
                        nc.vector.scalar_tensor_tensor(
                            out=ps_ov[nk][:], in0=ps_ov[nk][:],
                            scalar=rr[:, nk:nk + 1], in1=gnc[:, nk, :],
                            op0=MULT, op1=ADD)
                        if debug and c == 0 and nk == 0:
                            dpre = eB.tile([128, 512], F32, name="dpre")
                            nc.vector.tensor_copy(out=dpre, in_=ps_ov[nk][:])
                            nc.sync.dma_start(out=dbg["d_pre00"], in_=dpre)
                        st6 = eB.tile([128, 6], F32, tag=f"st{nk}", name=f"st{c}_{nk}")
                        nc.vector.bn_stats(out=st6, in_=ps_ov[nk][:])
                        # per-nk LN chain: no cross-nk barrier, so each nk's
                        # LN-apply + out DMA streams right behind its stats
                        mv2 = eB.tile([128, 2], F32, tag=f"mv{nk}",
                                      name=f"mv{c}_{nk}")
                        nc.vector.bn_aggr(out=mv2, in_=st6)
                        rstd1 = eB.tile([128, 1], F32, tag=f"rs{nk}",
                                        name=f"rs{c}_{nk}")
                        # rstd = (var+eps)^-0.5 on DVE: keeps ACT on the
                        # Exp/Identity table (no LoadActFuncSet thrash)
                        nc.vector.tensor_scalar(out=rstd1, in0=mv2[:, 1:2],
                                                scalar1=LN_EPS, scalar2=-0.5,
                                                op0=ADD, op1=mybir.AluOpType.pow)
                        nmr1 = eB.tile([128, 1], F32, tag=f"nm{nk}",
                                       name=f"nm{c}_{nk}")
                        nc.vector.scalar_tensor_tensor(out=nmr1, in0=mv2[:, 0:1],
                                                       scalar=-1.0, in1=rstd1,
                                                       op0=MULT, op1=MULT)
                        if use_gb:
                            yt = eB.tile([128, 512], F32, tag=f"yt{nk}",
                                         name=f"yt{c}_{nk}")
                            nc.scalar.activation(out=yt, in_=ps_ov[nk][:],
                                                 func=IDENT, bias=nmr1,
                                                 scale=rstd1)
                            nc.vector.tensor_mul(out=yt, in0=yt, in1=gam_bc)
                            nc.vector.tensor_add(out=ob[:, nk, :], in0=yt,
                                                 in1=bet_bc)
                        else:
                            nc.scalar.activation(out=ob[:, nk, :], in_=ps_ov[nk][:],
                                                 func=IDENT, bias=nmr1,
                                                 scale=rstd1)
                        # out DMA on the otherwise-idle Pool queue, one per nk
                        nc.gpsimd.dma_start(out=od[:, nk, :], in_=ob[:, nk, :])
    nc.compile()
    return nc


_CACHE = {}


def _get_nc(n_loc, n_cores, use_gb=False, debug=False):
    key = (n_loc, n_cores, use_gb, debug)
    if key not in _CACHE:
        _CACHE[key] = build(n_loc, n_cores, use_gb, debug)
    return _CACHE[key]


def kernel(g, g_p, W, Wq, bq, Wk, bk, Wv, bv, Wgp, bgp, Wkp, bkp, gamma, beta,
           _trace=False, _debug=False):
    g = np.asarray(g, np.float32)
    g_p = np.asarray(g_p, np.float32)
    Ws = np.asarray(W, np.float32).reshape(M, D)
    Wq = np.asarray(Wq, np.float32)
    Wk = np.asarray(Wk, np.float32)
    Wv = np.asarray(Wv, np.float32)
    Wgp = np.asarray(Wgp, np.float32)
    Wkp = np.asarray(Wkp, np.float32)
    bq = np.asarray(bq, np.float32)
    bk = np.asarray(bk, np.float32)
    bv = np.asarray(bv, np.float32)
    bgp = np.asarray(bgp, np.float32)
    bkp = np.asarray(bkp, np.float32)
    gamma = np.asarray(gamma, np.float32)
    beta = np.asarray(beta, np.float32)

    use_gb = not (np.all(gamma == 1.0) and np.all(beta == 0.0))
    n = g.shape[0]
    n_loc = n // N_CORES
    nc = _get_nc(n_loc, N_CORES, use_gb, _debug)

    # host-side weight precompute
    K0 = Ws @ Wk.T + bk + bkp                     # [M, D]
    V = Ws @ Wv.T + bv                            # [M, D]
    aq0 = SCALE * (Wq.T @ K0.T)                   # [D, M]
    agp0 = SCALE * (Wgp.T @ K0.T)                 # [D, M]
    GQ = SCALE * (Wkp.T @ Wq)                     # [D', D]
    GGP = SCALE * (Wkp.T @ Wgp)                   # [D', D]
    bs = bq + bgp
    c0 = SCALE * (bs @ K0.T)                      # [M]
    u0 = SCALE * (Wkp.T @ bs)                     # [D']

    bf = ml_dtypes.bfloat16
    f16 = np.float16
    shared = {
        "wst": np.ascontiguousarray(Ws.T),
        "v": V.astype(bf),
        "aq0": np.ascontiguousarray(aq0).astype(f16),
        "agp0": np.ascontiguousarray(agp0).astype(f16),
        "gq": GQ.astype(bf),
        "ggp": GGP.astype(bf),
        "gqt": np.ascontiguousarray(GQ.T).astype(f16),
        "ggpt": np.ascontiguousarray(GGP.T).astype(f16),
        "id128": np.eye(128, dtype=f16),
        "c0": c0, "u0": u0,
    }
    if use_gb:
        shared["gamma"] = gamma
        shared["beta"] = beta
    in_maps = []
    for cid in range(N_CORES):
        sl = slice(cid * n_loc, (cid + 1) * n_loc)
        gs = g[sl]
        gps = g_p[sl]
        in_maps.append({
            "gt": np.ascontiguousarray(gs.T).astype(f16),
            "gpt": np.ascontiguousarray(gps.T).astype(f16),
            "gpn": gps.astype(f16),
            "gn": gs.astype(f16),
            **shared,
        })
    res = bass_utils.run_bass_kernel_spmd(
        nc, in_maps, core_ids=list(range(N_CORES)), trace=_trace)
    out = np.concatenate([res.results[cid]["out"] for cid in range(N_CORES)], axis=0)
    if _debug:
        return out, res.results
    if _trace:
        return out, res
    return out



# revision 41
# speedup vs baseline: 1.0444x; 1.0444x over previous
"""Trainium2 Bass kernel for nn_CrossAttention (N=65536 gaussians, M=512 tokens, D=512).

Runs SPMD on 8 NeuronCores; N sharded across cores (n_loc=8192 rows each).

v2 design (vs v1 baseline at 679us):
  - Host precomputes all weight-derived matrices (V, aq0, agp0, GQ, GGP, c0, u0)
    and ships g/g_p pre-transposed, removing all device-side weight prep and
    all PE transposes.
  - gT is DMA'd once into a persistent SBUF stash (f32r, 128KB/partition) and
    reused by both the pooling pass and the attention pass.
  - Pooling accumulation (P = gp.T @ E, l = 1.T @ E) runs in bf16; the
    AllReduce payload [128, 2560] is bf16 (P.T tiles + l replicated).
  - LayerNorm rstd = Exp(-0.5 * Ln(var+eps)) keeps every ACT func in one
    activation table (no LoadActFuncSet thrash); final scale/shift happens on
    ACT via Identity(bias=-mu*rstd, scale=rstd).
  - Residual+scale fused into one DVE scalar_tensor_tensor writing PSUM
    in place.

Math (per core, n_loc rows):
  E = exp(g @ Ws.T - C_SHIFT)             [n, m]   (pool scores, fixed shift)
  P.T = gp.T @ E, l = 1.T @ E             -> AllReduce -> kplT = P.T / l
  aq_s = aq0 + GQ.T-contract kplT  (aq0 = SCALE*Wq.T@K0.T, K0 = Ws@Wk.T+bk+bkp)
  scoresT = aq_s.T-contract gT + agp_s.T-contract gpT   [m, n]
  ep = exp(scoresT + c),   c = c0 + u0-contract kplT
  OV = ep.T-contract V ; r = col-sums ; out = LN(OV/r + g) * gamma + beta
"""
import numpy as np
import ml_dtypes

import concourse.bass as bass
import concourse.tile as tile
from concourse import bacc, mybir, bass_utils


N_CORES = 8
N_FULL = 65536
D = 512
M = 512
SCALE = (D // 8) ** -0.5  # 0.125
LN_EPS = 1e-5
C_SHIFT = 115.0
Z_CHUNKS = 4   # chunks whose scores are prefactored (scores0 + Z) to fill the
               # AllReduce window with AR-independent PE work; each costs 64
               # matmuls of prework, and the window fits ~240
F32 = mybir.dt.float32
F32R = mybir.dt.float32r
BF16 = mybir.dt.bfloat16
F16 = mybir.dt.float16
EXP = mybir.ActivationFunctionType.Exp
LN_F = mybir.ActivationFunctionType.Ln
SQRT = mybir.ActivationFunctionType.Sqrt
IDENT = mybir.ActivationFunctionType.Identity
MULT = mybir.AluOpType.mult
ADD = mybir.AluOpType.add


def _bcast(ap, parts):
    """Partition-broadcast a [F]-shaped DRAM AP to [parts, F] for DMA."""
    return bass.AP(tensor=ap.tensor, offset=ap.offset, ap=[[0, parts], *ap.ap])


def build(n_loc=N_FULL // N_CORES, n_cores=N_CORES, use_gb=False, debug=False):
    nch = n_loc // 512     # chunks of 512 rows
    assert n_loc % 512 == 0

    nc = bacc.Bacc("TRN2", target_bir_lowering=False, debug=False,
                   num_devices=n_cores)
    # per-core sharded inputs
    gt_d = nc.dram_tensor("gt", [D, n_loc], F16, kind="ExternalInput").ap()
    gpt_d = nc.dram_tensor("gpt", [D, n_loc], F16, kind="ExternalInput").ap()
    gpn_d = nc.dram_tensor("gpn", [n_loc, D], F16, kind="ExternalInput").ap()
    gn_d = nc.dram_tensor("gn", [n_loc, D], F16, kind="ExternalInput").ap()
    # replicated (host-precomputed) weights
    wst_d = nc.dram_tensor("wst", [D, M], F32R, kind="ExternalInput").ap()
    v_d = nc.dram_tensor("v", [M, D], BF16, kind="ExternalInput").ap()
    aq0_d = nc.dram_tensor("aq0", [D, M], F16, kind="ExternalInput").ap()
    agp0_d = nc.dram_tensor("agp0", [D, M], F16, kind="ExternalInput").ap()
    gq_d = nc.dram_tensor("gq", [D, D], BF16, kind="ExternalInput").ap()
    ggp_d = nc.dram_tensor("ggp", [D, D], BF16, kind="ExternalInput").ap()
    gqt_d = nc.dram_tensor("gqt", [D, D], F16, kind="ExternalInput").ap()
    ggpt_d = nc.dram_tensor("ggpt", [D, D], F16, kind="ExternalInput").ap()
    id_d = nc.dram_tensor("id128", [128, 128], F16, kind="ExternalInput").ap()
    c0_d = nc.dram_tensor("c0", [M], F32, kind="ExternalInput").ap()
    u0_d = nc.dram_tensor("u0", [D], F32, kind="ExternalInput").ap()
    if use_gb:
        gam_d = nc.dram_tensor("gamma", [D], F32, kind="ExternalInput").ap()
        bet_d = nc.dram_tensor("beta", [D], F32, kind="ExternalInput").ap()
    out_d = nc.dram_tensor("out", [n_loc, D], F32, kind="ExternalOutput").ap()
    dbg = {}
    if debug:
        for nm, sh, dt in [("d_kplT", [128, 4, 512], F16),
                           ("d_aq", [128, 4, 512], F16),
                           ("d_agp", [128, 4, 512], F16), ("d_c", [128, 4], F32),
                           ("d_ep00", [128, 512], BF16), ("d_r0", [128, 4], F32),
                           ("d_pre00", [128, 512], F32),
                           ("d_sc00", [128, 512], F32),
                           ("d_ov00", [128, 512], F32),
                           ("d_rr0", [128, 4], F32),
                           ("d_ep01", [128, 512], BF16),
                           ("d_ep02", [128, 512], BF16),
                           ("d_ep03", [128, 512], BF16)]:
            dbg[nm] = nc.dram_tensor(nm, sh, dt, kind="ExternalOutput").ap()

    with tile.TileContext(nc) as tc:
        with (
            tc.tile_pool(name="wts", bufs=1) as wts,
            tc.tile_pool(name="ps", bufs=2, space="PSUM") as pps,
            tc.tile_pool(name="dram", bufs=1, space="DRAM") as dpool,
        ):
            # ---------- persistent tiles ----------
            K = min(Z_CHUNKS, nch)
            gstash = wts.tile([128, 4, n_loc], F16)    # g.T stash [d%128, d//128, n]
            v_sb = wts.tile([128, 4, D], BF16)         # V [m-part, mt, d]
            aq_s = wts.tile([128, 4, D], F16)          # SCALE*Wq.T@K.T [d, dt, m]
            agp_s = wts.tile([128, 4, D], F16)
            kplT = wts.tile([128, 4, 512], F16)        # pooled P.T/l [d'%128, dt, m]
            # spill space for the AR-window prework (scores0 / Z per Z-chunk)
            s0_sb = [wts.tile([128, 4, 512], F16, name=f"s0_{c}") for c in range(K)]
            z_sb = [wts.tile([128, 4, 512], F16, name=f"z_{c}") for c in range(K)]
            c0_sb = wts.tile([128, 4], F32)            # c0[m] as [m%128, mt]
            u0_sb = wts.tile([128, 4], F32)            # u0[d'] as [d'%128, dt]
            c_sb = wts.tile([128, 4], F32)
            ones_bf = wts.tile([128, 128], BF16)
            nc.vector.memset(ones_bf, 1.0)
            id_sb = wts.tile([128, 128], F16)   # identity: lets PE add a
            # [m, n]-layout tile into an open PSUM accumulation group
            eps_sb = wts.tile([128, 1], F32)
            nc.vector.memset(eps_sb, LN_EPS)
            negc_sb = wts.tile([128, 1], F32)
            nc.vector.memset(negc_sb, -C_SHIFT)
            if use_gb:
                gam_bc = wts.tile([128, D], F32)
                bet_bc = wts.tile([128, D], F32)
                nc.scalar.dma_start(out=gam_bc, in_=_bcast(gam_d, 128))
                nc.scalar.dma_start(out=bet_bc, in_=_bcast(bet_d, 128))

            # ---------- phase A: pooling partials ----------
            ctxA = tc.tile_pool(name="pA", bufs=1, space="PSUM")
            pA = ctxA.__enter__()
            ps_p = [pA.tile([128, 512], F32, tag=f"psp{i}", bufs=1,
                            name=f"ps_p{i}") for i in range(4)]
            ps_l = pA.tile([128, 512], F32, tag="psl", bufs=1, name="ps_l")
            with tc.tile_pool(name="sAw", bufs=1) as sAw:
                wst_sb = sAw.tile([128, 4, M], F32R)
                # halve the first loads so the first E matmul (dk=0) starts
                # early; finer splits lose to the ~0.65us per-DMA issue cost.
                # Order: wst h0, gt0 h0 (the dk<2 inputs), then the h1 halves.
                wst_r = wst_d.rearrange("(t p) m -> p t m", p=128)
                nc.sync.dma_start(out=wst_sb[:, 0:2, :], in_=wst_r[:, 0:2, :])
                with (tc.tile_pool(name="sA", bufs=2) as sA,
                      tc.tile_pool(name="etp", bufs=1) as etpool):
                    for c in range(nch):
                        nsl = slice(c * 512, (c + 1) * 512)
                        if c == 0:
                            gt_r = gt_d.rearrange("(t p) n -> p t n", p=128)
                            nc.sync.dma_start(out=gstash[:, 0:2, nsl],
                                              in_=gt_r[:, 0:2, nsl])
                            nc.sync.dma_start(out=wst_sb[:, 2:4, :],
                                              in_=wst_r[:, 2:4, :])
                            nc.sync.dma_start(out=gstash[:, 2:4, nsl],
                                              in_=gt_r[:, 2:4, nsl])
                        else:
                            nc.sync.dma_start(
                                out=gstash[:, :, nsl],
                                in_=gt_d.rearrange("(t p) n -> p t n", p=128)[:, :, nsl])
                        gpc = sA.tile([128, 4, D], F16, tag="gpc", name=f"gpc{c}")
                        nc.sync.dma_start(
                            out=gpc,
                            in_=gpn_d[nsl, :].rearrange("(j p) d -> p j d", p=128))
                        for j in range(4):
                            p_s = pps.tile([128, 512], F32, tag="s",
                                           name=f"psA{c}_{j}")
                            for dk in range(4):
                                nc.tensor.matmul(
                                    p_s[:],
                                    gstash[:, dk,
                                           c * 512 + j * 128:c * 512 + (j + 1) * 128],
                                    wst_sb[:, dk, :], start=(dk == 0), stop=(dk == 3))
                            et = etpool.tile([128, 512], F16, tag=f"et{j}",
                                         name=f"et{c}_{j}")
                            nc.scalar.activation(out=et, in_=p_s[:], func=EXP,
                                                 bias=negc_sb, scale=1.0)
                            first = (c == 0 and j == 0)
                            last = (c == nch - 1 and j == 3)
                            for d2t in range(4):
                                nc.tensor.matmul(
                                    ps_p[d2t][:],
                                    gpc[:, j, d2t * 128:(d2t + 1) * 128], et[:],
                                    start=first, stop=last)
                            nc.tensor.matmul(ps_l[:], ones_bf[:], et[:],
                                             start=first, stop=last)
                        if c == 0:
                            # weight loads deferred past the startup-critical
                            # wst + chunk-0 DMAs; needed only in phase B
                            nc.scalar.dma_start(
                                out=v_sb, in_=v_d.rearrange("(t p) d -> p t d", p=128))
                            nc.scalar.dma_start(
                                out=aq_s, in_=aq0_d.rearrange("(t p) m -> p t m", p=128))
                            nc.scalar.dma_start(
                                out=agp_s,
                                in_=agp0_d.rearrange("(t p) m -> p t m", p=128))
                            nc.scalar.dma_start(
                                out=c0_sb, in_=c0_d.rearrange("(t p) -> p t", p=128))
                            nc.scalar.dma_start(
                                out=u0_sb, in_=u0_d.rearrange("(t p) -> p t", p=128))

            # ---------- AllReduce of (P.T || l) in bf16 ----------
            # payload = P.T [128, 2048] + l [512] once (not 128x replicated):
            # 525,312 B -> (15000 + bytes/40) * 1.875 = 52.7us in the cost model
            NPL = 128 * 2048 + 512
            with tc.tile_pool(name="arp", bufs=1) as arp:
                # partial copies on DVE: ACT is still draining phase A's exps
                pl_sb = arp.tile([128, 4 * 512], BF16)
                for d2t in range(4):
                    nc.vector.tensor_copy(out=pl_sb[:, d2t * 512:(d2t + 1) * 512],
                                          in_=ps_p[d2t][:])
                l_row = arp.tile([1, 512], BF16)
                nc.vector.tensor_copy(out=l_row, in_=ps_l[0:1, :])
                ctxA.__exit__(None, None, None)
                ar_in = dpool.tile([NPL], BF16)
                ar_out = dpool.tile([NPL], BF16, addr_space="Shared")
                nc.gpsimd.dma_start(
                    out=ar_in[0:128 * 2048].rearrange("(p f) -> p f", p=128),
                    in_=pl_sb[:])
                nc.gpsimd.dma_start(
                    out=ar_in[128 * 2048:NPL].rearrange("(p f) -> p f", p=1),
                    in_=l_row[:])
                nc.gpsimd.collective_compute(
                    "AllReduce", mybir.AluOpType.add,
                    replica_groups=[list(range(n_cores))],
                    ins=[ar_in.opt()], outs=[ar_out.opt()])

                # ---------- AR-window prework (AR-independent) ----------
                # For the first K chunks compute scores0 = aq0.T@gT + agp0.T@gpT
                # and Z = GQ@gT + GGP@gpT while the collective runs.  Post-AR
                # those chunks only need Delta = kplT.T@Z (16 mm instead of 32).
                gqt_sb = arp.tile([128, 4, D], F16)
                ggpt_sb = arp.tile([128, 4, D], F16)
                nc.sync.dma_start(out=gqt_sb,
                                  in_=gqt_d.rearrange("(t p) e -> p t e", p=128))
                nc.sync.dma_start(out=ggpt_sb,
                                  in_=ggpt_d.rearrange("(t p) e -> p t e", p=128))
                nc.sync.dma_start(out=id_sb, in_=id_d)
                with tc.tile_pool(name="zin", bufs=2) as zin:
                    for c in range(K):
                        nsl = slice(c * 512, (c + 1) * 512)
                        gptz = zin.tile([128, 4, 512], F16, tag="gptz",
                                        name=f"gptz{c}")
                        nc.sync.dma_start(
                            out=gptz,
                            in_=gpt_d.rearrange("(t p) n -> p t n", p=128)[:, :, nsl])
                        for mt in range(4):
                            p_s0 = pps.tile([128, 512], F32, tag="s",
                                            name=f"zs{c}_{mt}")
                            for dk in range(4):
                                nc.tensor.matmul(
                                    p_s0[:], aq_s[:, dk, mt * 128:(mt + 1) * 128],
                                    gstash[:, dk, nsl], start=(dk == 0), stop=False)
                            for dk in range(4):
                                nc.tensor.matmul(
                                    p_s0[:], agp_s[:, dk, mt * 128:(mt + 1) * 128],
                                    gptz[:, dk, :], start=False, stop=(dk == 3))
                            nc.scalar.copy(out=s0_sb[c][:, mt, :], in_=p_s0[:])
                        for dt in range(4):
                            p_z = pps.tile([128, 512], F32, tag="s",
                                           name=f"zz{c}_{dt}")
                            for dk in range(4):
                                nc.tensor.matmul(
                                    p_z[:], gqt_sb[:, dk, dt * 128:(dt + 1) * 128],
                                    gstash[:, dk, nsl], start=(dk == 0), stop=False)
                            for dk in range(4):
                                nc.tensor.matmul(
                                    p_z[:], ggpt_sb[:, dk, dt * 128:(dt + 1) * 128],
                                    gptz[:, dk, :], start=False, stop=(dk == 3))
                            nc.scalar.copy(out=z_sb[c][:, dt, :], in_=p_z[:])

                # receive: l first (recip is on its critical path), then each
                # P.T quarter immediately followed by its kplT multiply, so
                # the first Delta matmul starts ~2.5us after AR completion
                lrep = arp.tile([128, 512], BF16)
                nc.sync.dma_start(out=lrep, in_=_bcast(ar_out[128 * 2048:NPL], 128))
                lrec = arp.tile([128, 512], F32)
                nc.vector.reciprocal(out=lrec, in_=lrep)
                plr_sb = arp.tile([128, 4 * 512], BF16)
                for dt in range(4):
                    nc.sync.dma_start(
                        out=plr_sb[:, dt * 512:(dt + 1) * 512],
                        in_=ar_out[dt * 128 * 512:(dt + 1) * 128 * 512].rearrange(
                            "(p f) -> p f", p=128))
                    nc.vector.tensor_mul(out=kplT[:, dt, :],
                                         in0=plr_sb[:, dt * 512:(dt + 1) * 512],
                                         in1=lrec)
                # c correction first: every ep exp needs the c bias
                u0b = arp.tile([128, 4], F16)
                nc.vector.tensor_copy(out=u0b, in_=u0_sb)
                with tc.tile_pool(name="pc", bufs=1, space="PSUM") as pcp:
                    p_c = pcp.tile([128, 4], F32, tag="pc", bufs=1, name="p_c")
                    for mt in range(4):
                        for di in range(4):
                            nc.tensor.matmul(
                                p_c[:, mt:mt + 1],
                                kplT[:, di, mt * 128:(mt + 1) * 128],
                                u0b[:, di:di + 1], start=(di == 0), stop=(di == 3))
                    nc.vector.tensor_add(out=c_sb, in0=c0_sb, in1=p_c[:])
                gq_sb = arp.tile([128, 4, D], BF16)
                ggp_sb = arp.tile([128, 4, D], BF16)
                nc.sync.dma_start(out=gq_sb,
                                  in_=gq_d.rearrange("(t p) d -> p t d", p=128))
                nc.sync.dma_start(out=ggp_sb,
                                  in_=ggp_d.rearrange("(t p) d -> p t d", p=128))
                for gmat, dst in ((gq_sb, aq_s), (ggp_sb, agp_s)):
                    for dt in range(4):
                        p_corr = pps.tile([128, 512], F32, tag="s",
                                          name=f"pc_{dst.tensor.name}_{dt}")
                        for di in range(4):
                            nc.tensor.matmul(
                                p_corr[:], gmat[:, di, dt * 128:(dt + 1) * 128],
                                kplT[:, di, :], start=(di == 0), stop=(di == 3))
                        nc.vector.tensor_add(out=dst[:, dt, :], in0=dst[:, dt, :],
                                             in1=p_corr[:])
                if debug:
                    nc.sync.dma_start(out=dbg["d_kplT"], in_=kplT)
                    nc.sync.dma_start(out=dbg["d_aq"], in_=aq_s)
                    nc.sync.dma_start(out=dbg["d_agp"], in_=agp_s)
                    nc.sync.dma_start(out=dbg["d_c"], in_=c_sb)

            # ---------- phase B: attention ----------
            with (tc.tile_pool(name="sB", bufs=3) as sB,
                  tc.tile_pool(name="epp", bufs=1) as epool,
                  tc.tile_pool(name="eB", bufs=2) as eB,
                  tc.tile_pool(name="oB", bufs=1) as oB,
                  tc.tile_pool(name="pB", bufs=1, space="PSUM") as pB):
                for c in range(nch):
                    nsl = slice(c * 512, (c + 1) * 512)
                    if c >= K:
                        gptc = sB.tile([128, 4, 512], F16, tag="gptc",
                                       name=f"gptc{c}")
                        nc.sync.dma_start(
                            out=gptc,
                            in_=gpt_d.rearrange("(t p) n -> p t n", p=128)[:, :, nsl])
                    gnc = sB.tile([128, 4, 512], F16, tag="gnc", name=f"gnc{c}")
                    nc.sync.dma_start(
                        out=gnc, in_=gn_d[nsl, :].rearrange("(j p) d -> p j d", p=128))
                    ps_ov = [pB.tile([128, 512], F32, tag=f"ov{k}", bufs=1,
                                     name=f"ov{c}_{k}") for k in range(4)]
                    ps_r = pB.tile([128, 4], F32, tag="r", bufs=1, name=f"r{c}")
                    eps = []
                    for mt in range(4):
                        p_sc = pps.tile([128, 512], F32, tag="s", name=f"sc{c}_{mt}")
                        if c < K:
                            # spilled scores0 via identity matmul first (needs
                            # no kplT), then Delta = kplT.T @ Z
                            nc.tensor.matmul(p_sc[:], id_sb[:],
                                             s0_sb[c][:, mt, :],
                                             start=True, stop=False)
                            for dt in range(4):
                                nc.tensor.matmul(
                                    p_sc[:], kplT[:, dt, mt * 128:(mt + 1) * 128],
                                    z_sb[c][:, dt, :], start=False, stop=(dt == 3))
                        else:
                            for dk in range(4):
                                nc.tensor.matmul(
                                    p_sc[:], aq_s[:, dk, mt * 128:(mt + 1) * 128],
                                    gstash[:, dk, nsl], start=(dk == 0), stop=False)
                            for dk in range(4):
                                nc.tensor.matmul(
                                    p_sc[:], agp_s[:, dk, mt * 128:(mt + 1) * 128],
                                    gptc[:, dk, :], start=False, stop=(dk == 3))
                        ep = epool.tile([128, 512], BF16, tag=f"ep{mt}",
                                     name=f"ep{c}_{mt}")
                        if debug and c == 0 and mt == 0:
                            dsc = eB.tile([128, 512], F32, name="dsc")
                            nc.vector.tensor_copy(out=dsc, in_=p_sc[:])
                            nc.sync.dma_start(out=dbg["d_sc00"], in_=dsc)
                        nc.scalar.activation(out=ep, in_=p_sc[:], func=EXP,
                                             bias=c_sb[:, mt:mt + 1], scale=1.0)
                        if debug and c == 0:
                            nc.sync.dma_start(out=dbg[f"d_ep0{mt}"], in_=ep)
                        eps.append(ep)
                        for nk in range(4):
                            nc.tensor.matmul(
                                ps_ov[nk][:], ep[:, nk * 128:(nk + 1) * 128],
                                v_sb[:, mt, :], start=(mt == 0), stop=(mt == 3))
                    # r groups must be column-sequential: start=True resets
                    # has_written for the whole bank, so interleaving the four
                    # column-groups of one tile drops earlier columns' first
                    # epoch.
                    for nk in range(4):
                        for mt in range(4):
                            nc.tensor.matmul(
                                ps_r[:, nk:nk + 1],
                                eps[mt][:, nk * 128:(nk + 1) * 128],
                                ones_bf[:, 0:1], start=(mt == 0), stop=(mt == 3))
                    rr = eB.tile([128, 4], F32, tag="rr", name=f"rr{c}")
                    if debug and c == 0:
                        dov = eB.tile([128, 512], F32, name="dov")
                        nc.vector.tensor_copy(out=dov, in_=ps_ov[0][:])
                        nc.sync.dma_start(out=dbg["d_ov00"], in_=dov)
                        drr = eB.tile([128, 4], F32, name="drr")
                        nc.vector.tensor_copy(out=drr, in_=ps_r[:])
                        nc.sync.dma_start(out=dbg["d_r0"], in_=drr)
                    nc.vector.reciprocal(out=rr, in_=ps_r[:])
                    if debug and c == 0:
                        nc.sync.dma_start(out=dbg["d_rr0"], in_=rr)
                    ob = oB.tile([128, 4, 512], F32, tag="ob", name=f"ob{c}")
                    od = out_d.rearrange("(c j p) d -> c p j d", p=128, j=4)[c]
                    for nk in range(4):
                        # pre = OV * (1/r) + g, in place in PSUM
                        nc.vector.scalar_tensor_tensor(
                            out=ps_ov[nk][:], in0=ps_ov[nk][:],
                            scalar=rr[:, nk:nk + 1], in1=gnc[:, nk, :],
                            op0=MULT, op1=ADD)
                        if debug and c == 0 and nk == 0:
                            dpre = eB.tile([128, 512], F32, name="dpre")
                            nc.vector.tensor_copy(out=dpre, in_=ps_ov[nk][:])
                            nc.sync.dma_start(out=dbg["d_pre00"], in_=dpre)
                        st6 = eB.tile([128, 6], F32, tag=f"st{nk}", name=f"st{c}_{nk}")
                        nc.vector.bn_stats(out=st6, in_=ps_ov[nk][:])
                        # per-nk LN chain: no cross-nk barrier, so each nk's
                        # LN-apply + out DMA streams right behind its stats
                        mv2 = eB.tile([128, 2], F32, tag=f"mv{nk}",
                                      name=f"mv{c}_{nk}")
                        nc.vector.bn_aggr(out=mv2, in_=st6)
                        rstd1 = eB.tile([128, 1], F32, tag=f"rs{nk}",
                                        name=f"rs{c}_{nk}")
                        # rstd = (var+eps)^-0.5 on DVE: keeps ACT on the
                        # Exp/Identity table (no LoadActFuncSet thrash)
                        nc.vector.tensor_scalar(out=rstd1, in0=mv2[:, 1:2],
                                                scalar1=LN_EPS, scalar2=-0.5,
                                                op0=ADD, op1=mybir.AluOpType.pow)
                        nmr1 = eB.tile([128, 1], F32, tag=f"nm{nk}",
                                       name=f"nm{c}_{nk}")
                        nc.vector.scalar_tensor_tensor(out=nmr1, in0=mv2[:, 0:1],
                                                       scalar=-1.0, in1=rstd1,
                                                       op0=MULT, op1=MULT)
                        if use_gb:
                            yt = eB.tile([128, 512], F32, tag=f"yt{nk}",
                                         name=f"yt{c}_{nk}")
                            nc.scalar.activation(out=yt, in_=ps_ov[nk][:],
                                                 func=IDENT, bias=nmr1,
                                                 scale=rstd1)
                            nc.vector.tensor_mul(out=yt, in0=yt, in1=gam_bc)
                            nc.vector.tensor_add(out=ob[:, nk, :], in0=yt,
                                                 in1=bet_bc)
                        else:
                            nc.scalar.activation(out=ob[:, nk, :], in_=ps_ov[nk][:],
                                                 func=IDENT, bias=nmr1,
                                                 scale=rstd1)
                        # out DMA on the otherwise-idle Pool queue, one per nk
                        nc.gpsimd.dma_start(out=od[:, nk, :], in_=ob[:, nk, :])
    nc.compile()
    return nc


_CACHE = {}


def _get_nc(n_loc, n_cores, use_gb=False, debug=False):
    key = (n_loc, n_cores, use_gb, debug)
    if key not in _CACHE:
        _CACHE[key] = build(n_loc, n_cores, use_gb, debug)
    return _CACHE[key]


def kernel(g, g_p, W, Wq, bq, Wk, bk, Wv, bv, Wgp, bgp, Wkp, bkp, gamma, beta,
           _trace=False, _debug=False):
    g = np.asarray(g, np.float32)
    g_p = np.asarray(g_p, np.float32)
    Ws = np.asarray(W, np.float32).reshape(M, D)
    Wq = np.asarray(Wq, np.float32)
    Wk = np.asarray(Wk, np.float32)
    Wv = np.asarray(Wv, np.float32)
    Wgp = np.asarray(Wgp, np.float32)
    Wkp = np.asarray(Wkp, np.float32)
    bq = np.asarray(bq, np.float32)
    bk = np.asarray(bk, np.float32)
    bv = np.asarray(bv, np.float32)
    bgp = np.asarray(bgp, np.float32)
    bkp = np.asarray(bkp, np.float32)
    gamma = np.asarray(gamma, np.float32)
    beta = np.asarray(beta, np.float32)

    use_gb = not (np.all(gamma == 1.0) and np.all(beta == 0.0))
    n = g.shape[0]
    n_loc = n // N_CORES
    nc = _get_nc(n_loc, N_CORES, use_gb, _debug)

    # host-side weight precompute
    K0 = Ws @ Wk.T + bk + bkp                     # [M, D]
    V = Ws @ Wv.T + bv                            # [M, D]
    aq0 = SCALE * (Wq.T @ K0.T)                   # [D, M]
    agp0 = SCALE * (Wgp.T @ K0.T)                 # [D, M]
    GQ = SCALE * (Wkp.T @ Wq)                     # [D', D]
    GGP = SCALE * (Wkp.T @ Wgp)                   # [D', D]
    bs = bq + bgp
    c0 = SCALE * (bs @ K0.T)                      # [M]
    u0 = SCALE * (Wkp.T @ bs)                     # [D']

    bf = ml_dtypes.bfloat16
    f16 = np.float16
    shared = {
        "wst": np.ascontiguousarray(Ws.T),
        "v": V.astype(bf),
        "aq0": np.ascontiguousarray(aq0).astype(f16),
        "agp0": np.ascontiguousarray(agp0).astype(f16),
        "gq": GQ.astype(bf),
        "ggp": GGP.astype(bf),
        "gqt": np.ascontiguousarray(GQ.T).astype(f16),
        "ggpt": np.ascontiguousarray(GGP.T).astype(f16),
        "id128": np.eye(128, dtype=f16),
        "c0": c0, "u0": u0,
    }
    if use_gb:
        shared["gamma"] = gamma
        shared["beta"] = beta
    in_maps = []
    for cid in range(N_CORES):
        sl = slice(cid * n_loc, (cid + 1) * n_loc)
        gs = g[sl]
        gps = g_p[sl]
        in_maps.append({
            "gt": np.ascontiguousarray(gs.T).astype(f16),
            "gpt": np.ascontiguousarray(gps.T).astype(f16),
            "gpn": gps.astype(f16),
            "gn": gs.astype(f16),
            **shared,
        })
    res = bass_utils.run_bass_kernel_spmd(
        nc, in_maps, core_ids=list(range(N_CORES)), trace=_trace)
    out = np.concatenate([res.results[cid]["out"] for cid in range(N_CORES)], axis=0)
    if _debug:
        return out, res.results
    if _trace:
        return out, res
    return out

